# revision 49
# baseline (speedup 1.0000x reference)
"""Causal self-attention (B=4, S=2048, D=1024, H=16) on 8 NeuronCores.

Sharding: core c handles batch b = c//2 and head-group g = c%2 (8 heads).
Each core computes qkv for its head group, causal attention for its 8 heads,
and a partial projection (its 512 rows of W_proj). Host sums the two partial
outputs per batch and adds b_proj.

Device layout notes:
 - x is passed transposed (xT [D, S]) and bf16; qT/kT are computed in
   [qkv_col, token] layout so the scores matmul needs no transposes:
   scoresT[k_tok, q_tok] = kT_tile.T @ qT  (lhsT = kT, contraction = head dim).
 - softmax runs on scoresT: exp on ScalarE (scale=1/8 folded in); the causal
   mask is applied in PSUM by adding a 0/-1e9 triangular tile to the diagonal
   128-wide band via an identity matmul (PE), and the masked prefix of each
   k-tile row is simply never computed or accumulated.
 - denominators come from a ones-column appended to v (v_aug [k,65]); the
   ctx matmul then yields [ctx(64 rows); sums(1 row)] per q block.
 - normalization: reciprocal of the sums row, partition-broadcast on GpSimd,
   one fused multiply+cast on VectorE.
"""

import numpy as np
import ml_dtypes

import concourse.bacc as bacc
import concourse.tile as tile
from concourse import mybir
from concourse.bass_utils import run_bass_kernel_spmd

BF16 = mybir.dt.bfloat16
F32 = mybir.dt.float32
EXP = mybir.ActivationFunctionType.Exp

B = 4
S = 2048  # tokens per batch
D = 1024
HG = 8    # heads per core
HD = 64
GC = HG * HD  # 512 qkv columns per core per q/k/v
N_CORES = 8
SCALE = 0.125  # 1/sqrt(64)


def _body(nc, xT, wq, wk, wv, wp, bqkv, tri, ident, outT, tc, layout="fill", use_bias=True):
    _const_cm = tc.tile_pool(name="const", bufs=1)
    const = _const_cm.__enter__()
    qT_sb = const.tile([128, 4, S], BF16)
    kT_sb = const.tile([128, 4, S], BF16)
    ctxT_sb = const.tile([128, 4, S], BF16)
    vaug_sb = const.tile([128, 16, HG, 65], BF16)
    wp_sb = const.tile([128, 4, D], BF16)
    tri_sb = const.tile([128, 128], BF16)
    ident_sb = const.tile([128, 128], BF16)
    b_sb = const.tile([1, 3 * GC], BF16)
    ones1 = const.tile([1, 512], BF16)

    m01_sb = const.tile([128, 128], BF16)

    nc.vector.memset(ones1[:], 1.0)
    nc.vector.memset(vaug_sb[:, :, :, 64:65], 1.0)
    nc.sync.dma_start(out=tri_sb[:], in_=tri.ap())
    nc.sync.dma_start(out=ident_sb[:], in_=ident.ap())
    if P_MASK == "dve":
        # 0/1 causal mask from tri (0 / -1e9): m01 = sign(tri) + 1
        nc.scalar.activation(m01_sb[:], tri_sb[:],
                             mybir.ActivationFunctionType.Sign)
        nc.scalar.add(m01_sb[:], m01_sb[:], 1.0)
    nc.sync.dma_start(out=b_sb[:], in_=bqkv.ap())
    for ct in range(4):
        nc.sync.dma_start(out=wp_sb[:, ct, :], in_=wp.ap()[128 * ct:128 * (ct + 1), :])

    xT_sb = const.tile([128, 8, S], BF16)
    wq_sb = const.tile([128, 8, GC], BF16)
    wk_sb = const.tile([128, 8, GC], BF16)
    wv_sb = const.tile([128, 8, GC], BF16)
    # xT + wq stream first so the first qk matmuls can start ASAP
    for t in range(8):
        nc.sync.dma_start(out=xT_sb[:, t, :], in_=xT.ap()[128 * t:128 * (t + 1), :])
        nc.sync.dma_start(out=wq_sb[:, t, :], in_=wq.ap()[128 * t:128 * (t + 1), :])
    for t in range(8):
        nc.sync.dma_start(out=wk_sb[:, t, :], in_=wk.ap()[128 * t:128 * (t + 1), :])
    for t in range(8):
        nc.sync.dma_start(out=wv_sb[:, t, :], in_=wv.ap()[128 * t:128 * (t + 1), :])

    # One shared PSUM pool scheme across all phases so emission can pipeline:
    #   scp "sc": [128,1024] slots x2 (4 banks) - qkv psums, scores, proj
    #   cxp "cx": [65,512] slots x4 (4 banks)   - ctx accumulators
    _scp_cm = tc.tile_pool(name="scp", bufs=2, space="PSUM")
    scp = _scp_cm.__enter__()
    _cxp_cm = tc.tile_pool(name="cxp", bufs=4, space="PSUM")
    cxp = _cxp_cm.__enter__()
    _prp_cm = tc.tile_pool(name="prp", bufs=6)
    prp = _prp_cm.__enter__()
    _nrm_cm = tc.tile_pool(name="nrm", bufs=4)
    nrm = _nrm_cm.__enter__()

    def qk_group(c, qk, tb):
        w_sb, dst, boff = ((wq_sb, qT_sb, 0), (wk_sb, kT_sb, GC))[qk]
        ps = scp.tile([128, 512], F32, tag="sc", name=f"qk_{c}_{boff}_{tb}")
        for t in range(8):
            nc.tensor.matmul(
                ps[:],
                lhsT=w_sb[:, t, 128 * c:128 * (c + 1)],
                rhs=xT_sb[:, t, 512 * tb:512 * (tb + 1)],
                start=(t == 0), stop=(not use_bias and t == 7))
        if use_bias:
            nc.tensor.matmul(
                ps[:],
                lhsT=b_sb[0:1, boff + 128 * c: boff + 128 * (c + 1)],
                rhs=ones1[0:1, :],
                start=False, stop=True)
        nc.vector.tensor_copy(dst[:, c, 512 * tb:512 * (tb + 1)], ps[:])

    def v_tile(j):
        # v in natural [token, v_col] layout, + bias, scattered into v_aug
        psv = scp.tile([128, 512], F32, tag="sc", name=f"pv_{j}")
        for t in range(8):
            nc.tensor.matmul(
                psv[:],
                lhsT=xT_sb[:, t, 128 * j:128 * (j + 1)],
                rhs=wv_sb[:, t, :],
                start=(t == 0), stop=(not use_bias and t == 7))
        if use_bias:
            nc.tensor.matmul(
                psv[:],
                lhsT=ones1[0:1, 0:128],
                rhs=b_sb[0:1, 2 * GC:3 * GC],
                start=False, stop=True)
        nc.vector.tensor_copy(
            vaug_sb[:, j, :, 0:64],
            psv[:].rearrange("p (h c) -> p h c", h=HG))

    def normalize(h, qb, ctx_ps):
        o = 64 * (h % 2)
        c = h // 2
        rec = nrm.tile([1, 512], F32, tag="rec", name=f"rec_{h}_{qb}")
        # NOTE: vector.reciprocal_approx_fast (custom DVE op) returns
        # garbage through the bass2jax HW path — do not use it here.
        if P_RECIP == "dma":
            # DVE reciprocal is 8 cyc/elem on ONE lane for a [1,512] row
            # (~4.3us); DMA-reshape to [128,4] puts it on 128 lanes
            srow = nrm.tile([1, 512], F32, tag="srow", name=f"sr_{h}_{qb}")
            nc.vector.tensor_copy(srow[:], ctx_ps[qb][64:65, :])
            zT = nrm.tile([128, 4], F32, tag="zt", name=f"zt_{h}_{qb}")
            nc.sync.dma_start(out=zT[:], in_=srow[:])
            zR = nrm.tile([128, 4], F32, tag="zr", name=f"zr_{h}_{qb}")
            nc.vector.reciprocal(zR[:], zT[:])
            nc.sync.dma_start(out=rec[:], in_=zR[:])
        else:
            nc.vector.reciprocal(rec[:], ctx_ps[qb][64:65, :])
        bc = nrm.tile([64, 512], F32, tag="bc", name=f"bc_{h}_{qb}")
        nc.gpsimd.partition_broadcast(bc[:], rec[:])
        if o == 0:
            nc.vector.tensor_mul(
                ctxT_sb[0:64, c, 512 * qb:512 * (qb + 1)],
                ctx_ps[qb][0:64, :], bc[:])
        else:
            stg = nrm.tile([64, 512], BF16, tag="stg", name=f"stg_{h}_{qb}")
            nc.vector.tensor_mul(stg[:], ctx_ps[qb][0:64, :], bc[:])
            nc.sync.dma_start(
                out=ctxT_sb[64:128, c, 512 * qb:512 * (qb + 1)], in_=stg[:])

    _ob3_cm = tc.tile_pool(name="ob3", bufs=4)
    ob3 = _ob3_cm.__enter__()

    def proj_group(m, tb):
        ps = scp.tile([128, 512], F32, tag="sc", name=f"p3_{m}_{tb}")
        for ct in range(4):
            nc.tensor.matmul(
                ps[:],
                lhsT=wp_sb[:, ct, 128 * m:128 * (m + 1)],
                rhs=ctxT_sb[:, ct, 512 * tb:512 * (tb + 1)],
                start=(ct == 0), stop=(ct == 3))
        ob = ob3.tile([128, 512], F32, tag="o3", name=f"ob_{m}_{tb}")
        if P_PROJ_EVAC == "act":
            # ACT is idle by the time the projection runs; keep DVE free
            nc.scalar.copy(ob[:], ps[:])
        else:
            # with proj interleaved into attention, ACT is busy with exps;
            # DVE has slack once the reciprocals are off its critical path
            nc.vector.tensor_copy(ob[:], ps[:])
        nc.sync.dma_start(
            out=outT.ap()[128 * m:128 * (m + 1), 512 * tb:512 * (tb + 1)],
            in_=ob[:])

    def h7_filler(j):
        # tb-block tb of the projection becomes legal once head 7's q-block
        # tb is normalized at j = 4*tb + 3; emit 2 (m, tb) groups per j
        if j >= 4:
            idx = j - 4
            tb, pair = idx // 4, idx % 4
            proj_group(2 * pair, tb)
            proj_group(2 * pair + 1, tb)

    def head_block(h, filler=None, pipe=False):
        o = 64 * (h % 2)
        c = h // 2
        ctx_ps = [cxp.tile([65, 512], F32, tag="cx", name=f"cx_{h}_{qb}")
                  for qb in range(4)]

        def scores_exp(j):
            qbm, r = divmod(j, 4)
            width = S - 512 * qbm
            rel0 = 128 * r
            pT = prp.tile([128, S], BF16, tag="probs", name=f"pT_{h}_{j}")
            # scores chunks of <=1024 free, one exp per chunk; the causal mask
            # is applied in PSUM by adding tri_neg (0 / -1e9) to the diagonal
            # 128-wide band via an identity matmul, keeping the whole
            # scores->exp chain on PE->ACT only
            for ch0 in range(0, width, 1024):
                ch1 = min(ch0 + 1024, width)
                lo = max(ch0, rel0)
                if lo >= ch1:
                    continue
                ps = scp.tile([128, 1024], F32, tag="sc", name=f"sc_{h}_{j}_{ch0}")
                for qb in range(qbm + ch0 // 512, qbm + ch1 // 512):
                    rq0 = (qb - qbm) * 512
                    mlo = max(rq0, rel0)
                    diag = mlo == rel0 and ch0 == 0 and P_MASK != "dve"
                    nc.tensor.matmul(
                        ps[:, mlo - ch0: rq0 + 512 - ch0],
                        lhsT=kT_sb[o:o + 64, c, 128 * j:128 * (j + 1)],
                        rhs=qT_sb[o:o + 64, c,
                                  512 * qbm + mlo: 512 * qbm + rq0 + 512],
                        start=True, stop=not diag, skip_group_check=True)
                    if diag:
                        nc.tensor.matmul(
                            ps[:, rel0 - ch0: rel0 - ch0 + 128],
                            lhsT=ident_sb[:],
                            rhs=tri_sb[:],
                            start=False, stop=True, skip_group_check=True)
                nc.scalar.activation(
                    pT[:, lo:ch1], ps[:, lo - ch0:ch1 - ch0], EXP, scale=SCALE)
            if P_MASK == "dve":
                # zero the strict upper triangle of the diagonal 128-block
                nc.vector.tensor_mul(
                    pT[:, rel0:rel0 + 128], pT[:, rel0:rel0 + 128], m01_sb[:])
            return pT

        def ctx_acc(j, pT):
            # ctx accumulation (with sums in row 64); the diagonal block's
            # masked prefix [0, rel0) is never computed nor accumulated
            qbm, r = divmod(j, 4)
            rel0 = 128 * r
            for qb in range(qbm, 4):
                lo = rel0 if qb == qbm else 0
                nc.tensor.matmul(
                    ctx_ps[qb][:, lo:512],
                    lhsT=vaug_sb[:, j, h, :],
                    rhs=pT[:, (qb - qbm) * 512 + lo: (qb - qbm + 1) * 512],
                    start=(j == 0), stop=(j == 4 * qb + 3),
                    skip_group_check=pipe)
            if r == 3:
                # qb = (j-3)//4 just received its last accumulation
                normalize(h, (j - 3) // 4, ctx_ps)

        if not pipe:
            for j in range(16):
                if filler is not None:
                    filler(j)
                pT = scores_exp(j)
                ctx_acc(j, pT)
        else:
            # 1-deep software pipeline: scores(j+1) is emitted before
            # ctx(j) so the PE never head-of-line blocks on exp(j)
            prev = None
            for j in range(16):
                if filler is not None:
                    filler(j)
                pT = scores_exp(j)
                if prev is not None:
                    ctx_acc(j - 1, prev)
                prev = pT
            ctx_acc(15, prev)

    def spread(groups):
        stride = max(1, 16 // max(1, len(groups)))
        def f(j):
            i = j // stride
            if j % stride == 0 and i < len(groups):
                groups[i]()
        return f

    qkg = [[(lambda c=c, qk=qk, tb=tb: qk_group(c, qk, tb))
            for qk in range(2) for tb in range(4)] for c in range(4)]
    if layout == "fill":
        # qk(0) upfront; v interleaved into h0 two iterations ahead of use;
        # qk(1..3) spread into h1..h5
        for g in qkg[0]:
            g()
        v_tile(0)
        v_tile(1)
        head_block(0, filler=lambda j: v_tile(j + 2) if j < 14 else None)
        head_block(1, filler=spread(qkg[1]))
        head_block(2, filler=spread(qkg[2][:4]))
        head_block(3, filler=spread(qkg[2][4:]))
        head_block(4, filler=spread(qkg[3][:4]))
        head_block(5, filler=spread(qkg[3][4:]))
        head_block(6)
        head_block(7, filler=h7_filler)
    elif layout == "pipe":
        # "fill" + 1-deep software pipeline inside each head block. The h7
        # proj filler shifts one j later: with the pipeline, normalize(qb)
        # is emitted during iteration j = 4*qb+4 (after the filler call),
        # so proj(tb) is only emission-legal from j = 4*tb+5 on.
        def h7_filler_pipe(j):
            if j >= 5:
                idx = j - 5
                tb, pair = idx // 4, idx % 4
                proj_group(2 * pair, tb)
                proj_group(2 * pair + 1, tb)

        for g in qkg[0]:
            g()
        v_tile(0)
        v_tile(1)
        head_block(0, filler=lambda j: v_tile(j + 2) if j < 14 else None,
                   pipe=True)
        head_block(1, filler=spread(qkg[1]), pipe=True)
        head_block(2, filler=spread(qkg[2][:4]), pipe=True)
        head_block(3, filler=spread(qkg[2][4:]), pipe=True)
        head_block(4, filler=spread(qkg[3][:4]), pipe=True)
        head_block(5, filler=spread(qkg[3][4:]), pipe=True)
        head_block(6, pipe=True)
        head_block(7, filler=h7_filler_pipe, pipe=True)
        proj_group(6, 2)
        proj_group(7, 2)
    elif layout == "seq":
        # all qkv upfront, then pure attention heads
        for c in range(4):
            for g in qkg[c]:
                g()
        for j in range(16):
            v_tile(j)
        for h in range(HG - 1):
            head_block(h)
        head_block(7, filler=h7_filler)
    elif layout == "block":
        # qkv blocks between head pairs
        for g in qkg[0]:
            g()
        for j in range(16):
            v_tile(j)
        for c in range(4):
            if c:
                for g in qkg[c]:
                    g()
            head_block(2 * c)
            head_block(2 * c + 1, filler=h7_filler if c == 3 else None)
    else:
        raise ValueError(layout)


    for pair in range(4):
        proj_group(2 * pair, 3)
        proj_group(2 * pair + 1, 3)

    _ob3_cm.__exit__(None, None, None)
    _nrm_cm.__exit__(None, None, None)
    _prp_cm.__exit__(None, None, None)
    _cxp_cm.__exit__(None, None, None)
    _scp_cm.__exit__(None, None, None)
    _const_cm.__exit__(None, None, None)


def _body_v2(nc, xT, wq, wk, wv, wp, tri, ident, outT, tc, state=None):
    """Pair-concurrent attention: heads (2c, 2c+1) processed together.

    Every matmul in the kernel is a K=64 row-tile instruction on PE tiles
    T0 (SBUF partitions 0:63) / T8 (64:127), so the PE never changes
    tiling mode, and back-to-back T0/T8 instructions targeting different
    PSUM banks execute concurrently (~2x stream rate):
      - qkv/proj: contraction split into even/odd 64-row chunks, T0/T8
        halves accumulate into the two halves of a [128,1024] PSUM slot,
        merged by one DVE add on evacuation.
      - scores: head A on T0 -> slot cols [0,512), head B on T8 -> cols
        [512,1024) (different banks); causal mask added to the diagonal
        128-block by split-identity x tri matmuls; ONE exp per (pair,qb,j)
        over a strided [128,2,512-lo] view.
      - ctx: 4 accumulators (A/B x lo/hi k-halves), each PSUM bank touched
        by exactly one PE row-tile; merged in the normalize step.
    Normalize: the softmax denominators are DMA-reshaped [1,1024]->[128,8]
    so the (8-cycle-per-element, single-lane) DVE reciprocal runs on 128
    lanes, then DMA'd back and partition-broadcast.
    Scheduling: qb-outer over q-blocks, pair-inner; qkv blocks for token
    range tb are emitted as PE fillers during qb=tb-1 (deadline-forced),
    proj blocks for tb during qb>=tb+1; ~1 filler per 2 attention steps
    hides the exp latency that the 2-slot scores ring exposes.
    """
    if state is None:
        _cms = [tc.tile_pool(name="const", bufs=1)]
        const = _cms[-1].__enter__()
        state = {
            "_cms": _cms,
            "xT_sb": const.tile([128, 8, S], BF16),
            "wq_sb": const.tile([128, 8, GC], BF16),
            "wk_sb": const.tile([128, 8, GC], BF16),
            "wv_sb": const.tile([128, 8, GC], BF16),
            "wp_sb": const.tile([128, 4, D], BF16),
            "tri_sb": const.tile([128, 128], BF16),
            "ident_sb": const.tile([128, 128], BF16),
            "qT_sb": const.tile([128, 4, S], BF16),
            "kT_sb": const.tile([128, 4, S], BF16),
            "vaug_sb": const.tile([128, 16, HG, 65], BF16),
            "ctxT_sb": const.tile([128, 4, S], BF16),
            "m01_sb": const.tile([128, 128], BF16),
        }
        for nm, mk in (
            ("scp", lambda: tc.tile_pool(name="scp", bufs=2, space="PSUM")),
            ("cxp", lambda: tc.tile_pool(name="cxp", bufs=1, space="PSUM")),
            ("pTp", lambda: tc.tile_pool(name="pTp", bufs=4)),
            ("stp", lambda: tc.tile_pool(name="stp", bufs=2)),
            ("nrm", lambda: tc.tile_pool(name="nrm", bufs=2)),
            ("obp", lambda: tc.tile_pool(name="obp", bufs=3)),
        ):
            cm = mk()
            _cms.append(cm)
            state[nm] = cm.__enter__()
    (xT_sb, wq_sb, wk_sb, wv_sb, wp_sb, tri_sb, ident_sb, qT_sb, kT_sb,
     vaug_sb, ctxT_sb, m01_sb) = (
        state[k] for k in ("xT_sb", "wq_sb", "wk_sb", "wv_sb", "wp_sb",
                           "tri_sb", "ident_sb", "qT_sb", "kT_sb",
                           "vaug_sb", "ctxT_sb", "m01_sb"))
    scp, cxp, pTp, stp, nrm, obp = (
        state[k] for k in ("scp", "cxp", "pTp", "stp", "nrm", "obp"))

    nc.vector.memset(vaug_sb[:, :, :, 64:65], 1.0)
    nc.sync.dma_start(out=tri_sb[:], in_=tri.ap())
    nc.sync.dma_start(out=ident_sb[:], in_=ident.ap())
    # 0/1 causal mask from tri (0 / -1e9): m01 = sign(tri) + 1
    nc.scalar.activation(m01_sb[:], tri_sb[:],
                         mybir.ActivationFunctionType.Sign)
    nc.scalar.add(m01_sb[:], m01_sb[:], 1.0)
    # k/q weights and the tb=0 slice of x first so attention can start ASAP
    for t in range(8):
        nc.sync.dma_start(out=wk_sb[:, t, :], in_=wk.ap()[128 * t:128 * (t + 1), :])
        nc.sync.dma_start(out=wq_sb[:, t, :], in_=wq.ap()[128 * t:128 * (t + 1), :])
    for tb in range(4):
        for t in range(8):
            nc.sync.dma_start(
                out=xT_sb[:, t, 512 * tb:512 * (tb + 1)],
                in_=xT.ap()[128 * t:128 * (t + 1), 512 * tb:512 * (tb + 1)])
        if tb == 0:
            for t in range(8):
                nc.sync.dma_start(
                    out=wv_sb[:, t, :], in_=wv.ap()[128 * t:128 * (t + 1), :])
    for ct in range(4):
        nc.sync.dma_start(out=wp_sb[:, ct, :], in_=wp.ap()[128 * ct:128 * (ct + 1), :])

    # qkv/proj fillers run as plain 128-contraction matmuls (a split-K
    # T0/T8 pair has the same wall time — stream-bound either way — but
    # would need a 2-PSUM-input merge, which DVE can't do in one op)
    def qk_block(which, c, tb):
        w_sb, dst = ((wq_sb, qT_sb), (wk_sb, kT_sb))[which]
        slot = scp.tile([128, 1024], F32, tag="sc", name=f"qk{which}_{c}_{tb}")
        cs = slice(128 * c, 128 * (c + 1))
        ts = slice(512 * tb, 512 * (tb + 1))
        for t in range(8):
            nc.tensor.matmul(
                slot[:, 0:512], lhsT=w_sb[:, t, cs], rhs=xT_sb[:, t, ts],
                start=(t == 0), stop=(t == 7), skip_group_check=True)
        nc.vector.tensor_copy(dst[:, c, ts], slot[:, 0:512])

    def v_block(j):
        slot = scp.tile([128, 1024], F32, tag="sc", name=f"v_{j}")
        js = slice(128 * j, 128 * (j + 1))
        for t in range(8):
            nc.tensor.matmul(
                slot[:, 0:512], lhsT=xT_sb[:, t, js], rhs=wv_sb[:, t, :],
                start=(t == 0), stop=(t == 7), skip_group_check=True)
        nc.vector.tensor_copy(
            vaug_sb[:, j, :, 0:64],
            slot[:, 0:512].rearrange("p (h c) -> p h c", h=HG))

    def proj_block(m, tb):
        slot = scp.tile([128, 1024], F32, tag="sc", name=f"p_{m}_{tb}")
        ms = slice(128 * m, 128 * (m + 1))
        ts = slice(512 * tb, 512 * (tb + 1))
        for ct in range(4):
            nc.tensor.matmul(
                slot[:, 0:512], lhsT=wp_sb[:, ct, ms], rhs=ctxT_sb[:, ct, ts],
                start=(ct == 0), stop=(ct == 3), skip_group_check=True)
        ob = obp.tile([128, 512], F32, tag="ob", name=f"ob_{m}_{tb}")
        nc.vector.tensor_copy(ob[:], slot[:, 0:512])
        nc.sync.dma_start(out=outT.ap()[ms, ts], in_=ob[:])

    # ---- filler queue: (ready_pos, deadline_pos, thunk); pos = 4*qb + c ----
    fillers = []
    for c in range(1, 4):
        fillers.append((0, c, lambda c=c: qk_block(1, c, 0)))
        fillers.append((0, c, lambda c=c: qk_block(0, c, 0)))
    for tb in range(1, 4):
        for c in range(4):
            fillers.append((4 * (tb - 1), 4 * tb + c,
                            lambda c=c, tb=tb: qk_block(1, c, tb)))
            fillers.append((4 * (tb - 1), 4 * tb + c,
                            lambda c=c, tb=tb: qk_block(0, c, tb)))
        for r in range(4):
            fillers.append((4 * (tb - 1), 4 * tb,
                            lambda j=4 * tb + r: v_block(j)))
    for tb in range(4):
        for m in range(8):
            fillers.append((4 * (tb + 1), 99,
                            lambda m=m, tb=tb: proj_block(m, tb)))

    def pop_filler(pos, force=False):
        for i, (rdy, dl, th) in enumerate(fillers):
            if rdy <= pos and (not force or dl <= pos):
                fillers.pop(i)
                th()
                return True
        return False

    def attn_block(c, qb, pos):
        n = 4 * qb + 4
        A, B = 2 * c, 2 * c + 1
        qs0 = 512 * qb
        accs = [cxp.tile([65, 512], F32, tag=t, name=f"{t}_{c}_{qb}")
                for t in ("calo", "cahi", "cblo", "cbhi")]

        def scores_exp(j):
            lo = 128 * (j - 4 * qb) if j >= 4 * qb else 0
            diag = (j >= 4 * qb) and V2_MASK != "dve"
            slot = scp.tile([128, 1024], F32, tag="sc", name=f"s_{c}_{qb}_{j}")
            js = slice(128 * j, 128 * (j + 1))
            nc.tensor.matmul(
                slot[:, lo:512], lhsT=kT_sb[0:64, c, js],
                rhs=qT_sb[0:64, c, qs0 + lo:qs0 + 512],
                start=True, stop=not diag, skip_group_check=True)
            nc.tensor.matmul(
                slot[:, 512 + lo:1024], lhsT=kT_sb[64:128, c, js],
                rhs=qT_sb[64:128, c, qs0 + lo:qs0 + 512],
                start=True, stop=not diag, skip_group_check=True)
            if diag and V2_MASK == "split":
                # add tri (0/-1e9) to the diagonal 128-block of both heads;
                # T0 pair first, then T8 pair (bank-overlap safe: a T8 mask
                # starts only after the same-bank T0 mask has finished)
                for h0, last in ((0, False), (64, True)):
                    nc.tensor.matmul(
                        slot[:, lo:lo + 128], lhsT=ident_sb[h0:h0 + 64, :],
                        rhs=tri_sb[h0:h0 + 64, :],
                        start=False, stop=last, skip_group_check=True)
                    nc.tensor.matmul(
                        slot[:, 512 + lo:512 + lo + 128],
                        lhsT=ident_sb[h0:h0 + 64, :], rhs=tri_sb[h0:h0 + 64, :],
                        start=False, stop=last, skip_group_check=True)
            elif diag:
                for off, last in ((0, False), (512, True)):
                    nc.tensor.matmul(
                        slot[:, off + lo:off + lo + 128], lhsT=ident_sb[:],
                        rhs=tri_sb[:],
                        start=False, stop=last, skip_group_check=True)
            pT = pTp.tile([128, 1024], BF16, tag="pt", name=f"pT_{c}_{qb}_{j}")
            if V2_EXP == "fused" or lo == 0:
                vi = slot[:].rearrange("p (b w) -> p b w", b=2)[:, :, lo:512]
                vo = pT[:].rearrange("p (b w) -> p b w", b=2)[:, :, lo:512]
                nc.scalar.activation(vo, vi, EXP, scale=SCALE)
            else:
                nc.scalar.activation(pT[:, lo:512], slot[:, lo:512],
                                     EXP, scale=SCALE)
                nc.scalar.activation(pT[:, 512 + lo:1024],
                                     slot[:, 512 + lo:1024], EXP, scale=SCALE)
            if j >= 4 * qb and V2_MASK == "dve":
                # zero the strict upper triangle of the diagonal 128-block
                # (probs layout is [k, q]: valid iff q >= k)
                nc.vector.tensor_mul(
                    pT[:, lo:lo + 128], pT[:, lo:lo + 128], m01_sb[:])
                nc.vector.tensor_mul(
                    pT[:, 512 + lo:512 + lo + 128],
                    pT[:, 512 + lo:512 + lo + 128], m01_sb[:])
            return pT, lo

        def ctx(j, pT, lo):
            first, last = (j == 0), (j == n - 1)
            if V2_CTX == "split":
                for i, (h, p0) in enumerate(((A, 0), (A, 64), (B, 0), (B, 64))):
                    off = 0 if h == A else 512
                    nc.tensor.matmul(
                        accs[i][:, lo:512],
                        lhsT=vaug_sb[p0:p0 + 64, j, h, :],
                        rhs=pT[p0:p0 + 64, off + lo:off + 512],
                        start=first, stop=last, skip_group_check=True)
            else:
                for i, h, off in ((0, A, 0), (2, B, 512)):
                    nc.tensor.matmul(
                        accs[i][:, lo:512],
                        lhsT=vaug_sb[:, j, h, :],
                        rhs=pT[:, off + lo:off + 512],
                        start=first, stop=last, skip_group_check=True)

        prev = None
        for j in range(n):
            if j % 2 == 1:
                pop_filler(pos)
            pT, lo = scores_exp(j)
            if prev is not None:
                ctx(*prev)
            prev = (j, pT, lo)
        # drain: one filler covers the last exp's latency, then final ctx
        pop_filler(pos)
        ctx(*prev)

        # ---- normalize both heads of the pair for this q-block ----
        st = stp.tile([65, 1024], F32, tag="st", name=f"st_{c}_{qb}")
        # DVE can read only ONE PSUM operand per op: copy then add
        nc.vector.tensor_copy(st[:, 0:512], accs[0][:])
        nc.vector.tensor_copy(st[:, 512:1024], accs[2][:])
        if V2_CTX == "split":
            nc.vector.tensor_add(st[:, 0:512], st[:, 0:512], accs[1][:])
            nc.vector.tensor_add(st[:, 512:1024], st[:, 512:1024], accs[3][:])
        rec = nrm.tile([1, 1024], F32, tag="rec", name=f"rec_{c}_{qb}")
        if V2_RECIP == "dma":
            # reciprocal of the [1,1024] sums row is 8 cyc/elem on ONE
            # lane; DMA-reshape to [128,8] puts it on 128 lanes
            zT = nrm.tile([128, 8], F32, tag="zt", name=f"zt_{c}_{qb}")
            nc.sync.dma_start(out=zT[:], in_=st[64:65, :])
            zR = nrm.tile([128, 8], F32, tag="zr", name=f"zr_{c}_{qb}")
            nc.vector.reciprocal(zR[:], zT[:])
            nc.sync.dma_start(out=rec[:], in_=zR[:])
        else:
            nc.vector.reciprocal(rec[:], st[64:65, :])
        bc = nrm.tile([64, 1024], F32, tag="bc", name=f"bc_{c}_{qb}")
        nc.gpsimd.partition_broadcast(bc[:], rec[:])
        cols = slice(qs0, qs0 + 512)
        nc.vector.tensor_mul(ctxT_sb[0:64, c, cols], st[0:64, 0:512], bc[:, 0:512])
        stgb = nrm.tile([64, 512], BF16, tag="stgb", name=f"stgb_{c}_{qb}")
        nc.vector.tensor_mul(stgb[:], st[0:64, 512:1024], bc[:, 512:1024])
        nc.sync.dma_start(out=ctxT_sb[64:128, c, cols], in_=stgb[:])

    # upfront work: k/q for pair 0 and all tb=0 v tiles
    qk_block(1, 0, 0)
    qk_block(0, 0, 0)
    for j in range(4):
        v_block(j)

    for qb in range(4):
        for c in range(4):
            pos = 4 * qb + c
            if (qb, c) != (0, 0):
                # k/q for this pair's q-block + anything past deadline
                while pop_filler(pos, force=True):
                    pass
                pop_filler(pos)
            attn_block(c, qb, pos)
    while pop_filler(99):
        pass

    return state


def _v2_close(state):
    for cm in reversed(state["_cms"]):
        cm.__exit__(None, None, None)


_CACHED = {}


def _build(reps=1, layout="fill", use_bias=True):
    key = (reps, layout, use_bias, V2_RECIP, V2_EXP, V2_CTX, V2_MASK,
           P_MASK, P_RECIP)
    if key in _CACHED:
        return _CACHED[key]
    nc = bacc.Bacc()
    xT = nc.dram_tensor("xT", [D, S], BF16, kind="ExternalInput")
    wq = nc.dram_tensor("wq", [D, GC], BF16, kind="ExternalInput")
    wk = nc.dram_tensor("wk", [D, GC], BF16, kind="ExternalInput")
    wv = nc.dram_tensor("wv", [D, GC], BF16, kind="ExternalInput")
    wp = nc.dram_tensor("wp", [GC, D], BF16, kind="ExternalInput")
    bqkv = None
    if layout not in ("v2", "p3"):
        bqkv = nc.dram_tensor("bqkv", [1, 3 * GC], BF16, kind="ExternalInput")
    tri = nc.dram_tensor("tri", [128, 128], BF16, kind="ExternalInput")
    ident = nc.dram_tensor("ident", [128, 128], BF16, kind="ExternalInput")
    outT = nc.dram_tensor("outT", [D, S], F32, kind="ExternalOutput")
    with tile.TileContext(nc) as tc:
        st = None
        for _ in range(reps):
            if layout == "v2":
                st = _body_v2(nc, xT, wq, wk, wv, wp, tri, ident, outT, tc,
                              state=st)
            else:
                _body(nc, xT, wq, wk, wv, wp, bqkv, tri, ident, outT, tc,
                      layout=layout, use_bias=use_bias)
        if st is not None:
            _v2_close(st)
    nc.compile()
    _CACHED[key] = nc
    return nc


def make_in_maps(x, W_attn, b_attn, W_proj):
    bf = ml_dtypes.bfloat16
    tri_np = np.where(np.arange(128)[None, :] >= np.arange(128)[:, None],
                      np.float32(0.0), np.float32(-1e9)).astype(bf)
    ident_np = np.eye(128, dtype=np.float32).astype(bf)
    in_maps = []
    for core in range(N_CORES):
        b, g = divmod(core, 2)
        cols = slice(GC * g, GC * (g + 1))
        in_maps.append({
            "xT": np.ascontiguousarray(x[b].T).astype(bf),
            "wq": np.ascontiguousarray(W_attn[:, cols]).astype(bf),
            "wk": np.ascontiguousarray(W_attn[:, D:][:, cols]).astype(bf),
            "wv": np.ascontiguousarray(W_attn[:, 2 * D:][:, cols]).astype(bf),
            "wp": np.ascontiguousarray(W_proj[cols, :]).astype(bf),
            "bqkv": np.concatenate(
                [b_attn[cols], b_attn[D:][cols], b_attn[2 * D:][cols]]
            ).reshape(1, 3 * GC).astype(bf),
            "tri": tri_np,
            "ident": ident_np,
        })
    return in_maps


LAYOUT = "pipe"  # current best layout; kernel() and test.py both use it
P_MASK = "pe"    # "pe" (ident x tri matmul) | "dve" (0/1 multiply on pT)
P_RECIP = "dve"  # "dve" (1-lane reciprocal) | "dma" (128-lane via reshape)
P_PROJ_EVAC = "act"  # "act" (scalar.copy) | "dve" (vector.tensor_copy)
V2_RECIP = "dma"   # "dma" (128-lane via DMA reshape) | "dve" (1-lane direct)
V2_EXP = "fused"   # "fused" (one strided exp per pair step) | "split"
V2_CTX = "split"   # "split" (4x 64-contract T0/T8) | "full" (2x 128-contract)
# "dve": multiply pT's diagonal block by a 0/1 mask on DVE (no PE mask work)
# "full": baseline-style 128-contract ident x tri matmul into PSUM
# "split": ident halves T0/T8 — BROKEN on HW (mixing PE row-tiles inside
#          one PSUM accumulation group fails device execution)
V2_MASK = "dve"


def kernel(x, W_attn, b_attn, W_proj, b_proj, _run_kwargs=None):
    x = np.asarray(x)
    W_attn = np.asarray(W_attn)
    b_attn = np.asarray(b_attn)
    W_proj = np.asarray(W_proj)
    b_proj = np.asarray(b_proj)

    use_bias = bool(np.any(b_attn))
    layout = "fill" if (use_bias and LAYOUT in ("v2", "p3")) else LAYOUT
    nc = _build(layout=layout, use_bias=use_bias)
    in_maps = make_in_maps(x, W_attn, b_attn, W_proj)
    declared = {
        alloc.memorylocations[0].name
        for alloc in nc.m.functions[0].allocations
        if isinstance(alloc, mybir.MemoryLocationSet)
        and alloc.kind == "ExternalInput"
    }
    in_maps = [{k: v for k, v in m.items() if k in declared} for m in in_maps]

    res = run_bass_kernel_spmd(
        nc, in_maps, core_ids=list(range(N_CORES)), **(_run_kwargs or {}))

    out = np.empty((B, S, D), np.float32)
    for b in range(B):
        acc = res.results[2 * b]["outT"] + res.results[2 * b + 1]["outT"]
        out[b] = acc.T + b_proj[None, :].astype(np.float32)
    if _run_kwargs:
        kernel.last_results = res
    return out



# revision 51
# speedup vs baseline: 1.3496x; 1.3496x over previous
"""Causal self-attention (B=4, S=2048, D=1024, H=16) on 8 NeuronCores.

Sharding: core c handles batch b = c//2 and head-group g = c%2 (8 heads).
Each core computes qkv for its head group, causal attention for its 8 heads,
and a partial projection (its 512 rows of W_proj). Host sums the two partial
outputs per batch and adds b_proj.

Device layout notes:
 - x is passed transposed (xT [D, S]) and bf16; qT/kT are computed in
   [qkv_col, token] layout so the scores matmul needs no transposes:
   scoresT[k_tok, q_tok] = kT_tile.T @ qT  (lhsT = kT, contraction = head dim).
 - softmax runs on scoresT: exp on ScalarE (scale=1/8 folded in); the causal
   mask is applied in PSUM by adding a 0/-1e9 triangular tile to the diagonal
   128-wide band via an identity matmul (PE), and the masked prefix of each
   k-tile row is simply never computed or accumulated.
 - denominators come from a ones-column appended to v (v_aug [k,65]); the
   ctx matmul then yields [ctx(64 rows); sums(1 row)] per q block.
 - normalization: reciprocal of the sums row, partition-broadcast on GpSimd,
   one fused multiply+cast on VectorE.
"""

import numpy as np
import ml_dtypes

import concourse.bacc as bacc
import concourse.tile as tile
from concourse import mybir
from concourse.bass_utils import run_bass_kernel_spmd

BF16 = mybir.dt.bfloat16
F32 = mybir.dt.float32
EXP = mybir.ActivationFunctionType.Exp

B = 4
S = 2048  # tokens per batch
D = 1024
HG = 8    # heads per core
HD = 64
GC = HG * HD  # 512 qkv columns per core per q/k/v
N_CORES = 8
SCALE = 0.125  # 1/sqrt(64)


def _body(nc, xT, wq, wk, wv, wp, bqkv, tri, ident, outT, tc, layout="fill", use_bias=True):
    _const_cm = tc.tile_pool(name="const", bufs=1)
    const = _const_cm.__enter__()
    qT_sb = const.tile([128, 4, S], BF16)
    kT_sb = const.tile([128, 4, S], BF16)
    ctxT_sb = const.tile([128, 4, S], BF16)
    vaug_sb = const.tile([128, 16, HG, 65], BF16)
    wp_sb = const.tile([128, 4, D], BF16)
    tri_sb = const.tile([128, 128], BF16)
    ident_sb = const.tile([128, 128], BF16)
    b_sb = const.tile([1, 3 * GC], BF16)
    ones1 = const.tile([1, 512], BF16)

    m01_sb = const.tile([128, 128], BF16)

    nc.vector.memset(ones1[:], 1.0)
    nc.vector.memset(vaug_sb[:, :, :, 64:65], 1.0)
    nc.sync.dma_start(out=tri_sb[:], in_=tri.ap())
    nc.sync.dma_start(out=ident_sb[:], in_=ident.ap())
    if P_MASK == "dve":
        # 0/1 causal mask from tri (0 / -1e9): m01 = sign(tri) + 1
        nc.scalar.activation(m01_sb[:], tri_sb[:],
                             mybir.ActivationFunctionType.Sign)
        nc.scalar.add(m01_sb[:], m01_sb[:], 1.0)
    nc.sync.dma_start(out=b_sb[:], in_=bqkv.ap())
    for ct in range(4):
        nc.sync.dma_start(out=wp_sb[:, ct, :], in_=wp.ap()[128 * ct:128 * (ct + 1), :])

    xT_sb = const.tile([128, 8, S], BF16)
    wq_sb = const.tile([128, 8, GC], BF16)
    wk_sb = const.tile([128, 8, GC], BF16)
    wv_sb = const.tile([128, 8, GC], BF16)
    # xT + wq stream first so the first qk matmuls can start ASAP
    for t in range(8):
        nc.sync.dma_start(out=xT_sb[:, t, :], in_=xT.ap()[128 * t:128 * (t + 1), :])
        nc.sync.dma_start(out=wq_sb[:, t, :], in_=wq.ap()[128 * t:128 * (t + 1), :])
    for t in range(8):
        nc.sync.dma_start(out=wk_sb[:, t, :], in_=wk.ap()[128 * t:128 * (t + 1), :])
    for t in range(8):
        nc.sync.dma_start(out=wv_sb[:, t, :], in_=wv.ap()[128 * t:128 * (t + 1), :])

    # One shared PSUM pool scheme across all phases so emission can pipeline:
    #   scp "sc": [128,1024] slots x2 (4 banks) - qkv psums, scores, proj
    #   cxp "cx": [65,512] slots x4 (4 banks)   - ctx accumulators
    _scp_cm = tc.tile_pool(name="scp", bufs=2, space="PSUM")
    scp = _scp_cm.__enter__()
    _cxp_cm = tc.tile_pool(name="cxp", bufs=4, space="PSUM")
    cxp = _cxp_cm.__enter__()
    _prp_cm = tc.tile_pool(name="prp", bufs=6)
    prp = _prp_cm.__enter__()
    _nrm_cm = tc.tile_pool(name="nrm", bufs=4)
    nrm = _nrm_cm.__enter__()

    def qk_group(c, qk, tb):
        w_sb, dst, boff = ((wq_sb, qT_sb, 0), (wk_sb, kT_sb, GC))[qk]
        ps = scp.tile([128, 512], F32, tag="sc", name=f"qk_{c}_{boff}_{tb}")
        for t in range(8):
            nc.tensor.matmul(
                ps[:],
                lhsT=w_sb[:, t, 128 * c:128 * (c + 1)],
                rhs=xT_sb[:, t, 512 * tb:512 * (tb + 1)],
                start=(t == 0), stop=(not use_bias and t == 7))
        if use_bias:
            nc.tensor.matmul(
                ps[:],
                lhsT=b_sb[0:1, boff + 128 * c: boff + 128 * (c + 1)],
                rhs=ones1[0:1, :],
                start=False, stop=True)
        nc.vector.tensor_copy(dst[:, c, 512 * tb:512 * (tb + 1)], ps[:])

    def v_tile(j):
        # v in natural [token, v_col] layout, + bias, scattered into v_aug
        psv = scp.tile([128, 512], F32, tag="sc", name=f"pv_{j}")
        for t in range(8):
            nc.tensor.matmul(
                psv[:],
                lhsT=xT_sb[:, t, 128 * j:128 * (j + 1)],
                rhs=wv_sb[:, t, :],
                start=(t == 0), stop=(not use_bias and t == 7))
        if use_bias:
            nc.tensor.matmul(
                psv[:],
                lhsT=ones1[0:1, 0:128],
                rhs=b_sb[0:1, 2 * GC:3 * GC],
                start=False, stop=True)
        nc.vector.tensor_copy(
            vaug_sb[:, j, :, 0:64],
            psv[:].rearrange("p (h c) -> p h c", h=HG))

    def normalize(h, qb, ctx_ps):
        o = 64 * (h % 2)
        c = h // 2
        rec = nrm.tile([1, 512], F32, tag="rec", name=f"rec_{h}_{qb}")
        # NOTE: vector.reciprocal_approx_fast (custom DVE op) returns
        # garbage through the bass2jax HW path — do not use it here.
        if P_RECIP == "dma":
            # DVE reciprocal is 8 cyc/elem on ONE lane for a [1,512] row
            # (~4.3us); DMA-reshape to [128,4] puts it on 128 lanes
            srow = nrm.tile([1, 512], F32, tag="srow", name=f"sr_{h}_{qb}")
            nc.vector.tensor_copy(srow[:], ctx_ps[qb][64:65, :])
            zT = nrm.tile([128, 4], F32, tag="zt", name=f"zt_{h}_{qb}")
            nc.sync.dma_start(out=zT[:], in_=srow[:])
            zR = nrm.tile([128, 4], F32, tag="zr", name=f"zr_{h}_{qb}")
            nc.vector.reciprocal(zR[:], zT[:])
            nc.sync.dma_start(out=rec[:], in_=zR[:])
        else:
            nc.vector.reciprocal(rec[:], ctx_ps[qb][64:65, :])
        bc = nrm.tile([64, 512], F32, tag="bc", name=f"bc_{h}_{qb}")
        nc.gpsimd.partition_broadcast(bc[:], rec[:])
        if o == 0:
            nc.vector.tensor_mul(
                ctxT_sb[0:64, c, 512 * qb:512 * (qb + 1)],
                ctx_ps[qb][0:64, :], bc[:])
        else:
            stg = nrm.tile([64, 512], BF16, tag="stg", name=f"stg_{h}_{qb}")
            nc.vector.tensor_mul(stg[:], ctx_ps[qb][0:64, :], bc[:])
            nc.sync.dma_start(
                out=ctxT_sb[64:128, c, 512 * qb:512 * (qb + 1)], in_=stg[:])

    _ob3_cm = tc.tile_pool(name="ob3", bufs=4)
    ob3 = _ob3_cm.__enter__()

    def proj_group(m, tb):
        ps = scp.tile([128, 512], F32, tag="sc", name=f"p3_{m}_{tb}")
        for ct in range(4):
            nc.tensor.matmul(
                ps[:],
                lhsT=wp_sb[:, ct, 128 * m:128 * (m + 1)],
                rhs=ctxT_sb[:, ct, 512 * tb:512 * (tb + 1)],
                start=(ct == 0), stop=(ct == 3))
        ob = ob3.tile([128, 512], F32, tag="o3", name=f"ob_{m}_{tb}")
        if P_PROJ_EVAC == "act":
            # ACT is idle by the time the projection runs; keep DVE free
            nc.scalar.copy(ob[:], ps[:])
        else:
            # with proj interleaved into attention, ACT is busy with exps;
            # DVE has slack once the reciprocals are off its critical path
            nc.vector.tensor_copy(ob[:], ps[:])
        nc.sync.dma_start(
            out=outT.ap()[128 * m:128 * (m + 1), 512 * tb:512 * (tb + 1)],
            in_=ob[:])

    def h7_filler(j):
        # tb-block tb of the projection becomes legal once head 7's q-block
        # tb is normalized at j = 4*tb + 3; emit 2 (m, tb) groups per j
        if j >= 4:
            idx = j - 4
            tb, pair = idx // 4, idx % 4
            proj_group(2 * pair, tb)
            proj_group(2 * pair + 1, tb)

    def head_block(h, filler=None, pipe=False):
        o = 64 * (h % 2)
        c = h // 2
        ctx_ps = [cxp.tile([65, 512], F32, tag="cx", name=f"cx_{h}_{qb}")
                  for qb in range(4)]

        def scores_exp(j):
            qbm, r = divmod(j, 4)
            width = S - 512 * qbm
            rel0 = 128 * r
            pT = prp.tile([128, S], BF16, tag="probs", name=f"pT_{h}_{j}")
            # scores chunks of <=1024 free, one exp per chunk; the causal mask
            # is applied in PSUM by adding tri_neg (0 / -1e9) to the diagonal
            # 128-wide band via an identity matmul, keeping the whole
            # scores->exp chain on PE->ACT only
            for ch0 in range(0, width, 1024):
                ch1 = min(ch0 + 1024, width)
                lo = max(ch0, rel0)
                if lo >= ch1:
                    continue
                ps = scp.tile([128, 1024], F32, tag="sc", name=f"sc_{h}_{j}_{ch0}")
                for qb in range(qbm + ch0 // 512, qbm + ch1 // 512):
                    rq0 = (qb - qbm) * 512
                    mlo = max(rq0, rel0)
                    diag = mlo == rel0 and ch0 == 0 and P_MASK != "dve"
                    nc.tensor.matmul(
                        ps[:, mlo - ch0: rq0 + 512 - ch0],
                        lhsT=kT_sb[o:o + 64, c, 128 * j:128 * (j + 1)],
                        rhs=qT_sb[o:o + 64, c,
                                  512 * qbm + mlo: 512 * qbm + rq0 + 512],
                        start=True, stop=not diag, skip_group_check=True)
                    if diag:
                        nc.tensor.matmul(
                            ps[:, rel0 - ch0: rel0 - ch0 + 128],
                            lhsT=ident_sb[:],
                            rhs=tri_sb[:],
                            start=False, stop=True, skip_group_check=True)
                nc.scalar.activation(
                    pT[:, lo:ch1], ps[:, lo - ch0:ch1 - ch0], EXP, scale=SCALE)
            if P_MASK == "dve":
                # zero the strict upper triangle of the diagonal 128-block
                nc.vector.tensor_mul(
                    pT[:, rel0:rel0 + 128], pT[:, rel0:rel0 + 128], m01_sb[:])
            return pT

        def ctx_acc(j, pT):
            # ctx accumulation (with sums in row 64); the diagonal block's
            # masked prefix [0, rel0) is never computed nor accumulated
            qbm, r = divmod(j, 4)
            rel0 = 128 * r
            for qb in range(qbm, 4):
                lo = rel0 if qb == qbm else 0
                nc.tensor.matmul(
                    ctx_ps[qb][:, lo:512],
                    lhsT=vaug_sb[:, j, h, :],
                    rhs=pT[:, (qb - qbm) * 512 + lo: (qb - qbm + 1) * 512],
                    start=(j == 0), stop=(j == 4 * qb + 3),
                    skip_group_check=pipe)
            if r == 3:
                # qb = (j-3)//4 just received its last accumulation
                normalize(h, (j - 3) // 4, ctx_ps)

        if not pipe:
            for j in range(16):
                if filler is not None:
                    filler(j)
                pT = scores_exp(j)
                ctx_acc(j, pT)
        else:
            # lag-deep software pipeline: scores(j+lag..) are emitted
            # before ctx(j) so the PE never head-of-line blocks on exp(j)
            lag = int(pipe)
            hist = []
            for j in range(16):
                if filler is not None:
                    filler(j)
                hist.append(scores_exp(j))
                if j >= lag:
                    ctx_acc(j - lag, hist[j - lag])
            for j in range(16 - lag, 16):
                ctx_acc(j, hist[j])

    def spread(groups):
        stride = max(1, 16 // max(1, len(groups)))
        def f(j):
            i = j // stride
            if j % stride == 0 and i < len(groups):
                groups[i]()
        return f

    qkg = [[(lambda c=c, qk=qk, tb=tb: qk_group(c, qk, tb))
            for qk in range(2) for tb in range(4)] for c in range(4)]
    if layout == "fill":
        # qk(0) upfront; v interleaved into h0 two iterations ahead of use;
        # qk(1..3) spread into h1..h5
        for g in qkg[0]:
            g()
        v_tile(0)
        v_tile(1)
        head_block(0, filler=lambda j: v_tile(j + 2) if j < 14 else None)
        head_block(1, filler=spread(qkg[1]))
        head_block(2, filler=spread(qkg[2][:4]))
        head_block(3, filler=spread(qkg[2][4:]))
        head_block(4, filler=spread(qkg[3][:4]))
        head_block(5, filler=spread(qkg[3][4:]))
        head_block(6)
        head_block(7, filler=h7_filler)
    elif layout in ("pipe", "pipe2"):
        # "fill" + lag-deep software pipeline inside each head block. The
        # h7 proj filler shifts lag iterations later: normalize(qb) is
        # emitted during iteration j = 4*qb+3+lag (after the filler call),
        # so proj(tb) is only emission-legal from j = 4*tb+4+lag on.
        lag = 1 if layout == "pipe" else 2

        def h7_filler_pipe(j):
            if j >= 4 + lag:
                idx = j - 4 - lag
                tb, pair = idx // 4, idx % 4
                proj_group(2 * pair, tb)
                proj_group(2 * pair + 1, tb)

        for g in qkg[0]:
            g()
        v_tile(0)
        v_tile(1)
        head_block(0, filler=lambda j: v_tile(j + 2) if j < 14 else None,
                   pipe=lag)
        head_block(1, filler=spread(qkg[1]), pipe=lag)
        head_block(2, filler=spread(qkg[2][:4]), pipe=lag)
        head_block(3, filler=spread(qkg[2][4:]), pipe=lag)
        head_block(4, filler=spread(qkg[3][:4]), pipe=lag)
        head_block(5, filler=spread(qkg[3][4:]), pipe=lag)
        head_block(6, pipe=lag)
        head_block(7, filler=h7_filler_pipe, pipe=lag)
        for i in range(lag):
            proj_group(6 - 2 * i, 2)
            proj_group(7 - 2 * i, 2)
    elif layout == "seq":
        # all qkv upfront, then pure attention heads
        for c in range(4):
            for g in qkg[c]:
                g()
        for j in range(16):
            v_tile(j)
        for h in range(HG - 1):
            head_block(h)
        head_block(7, filler=h7_filler)
    elif layout == "block":
        # qkv blocks between head pairs
        for g in qkg[0]:
            g()
        for j in range(16):
            v_tile(j)
        for c in range(4):
            if c:
                for g in qkg[c]:
                    g()
            head_block(2 * c)
            head_block(2 * c + 1, filler=h7_filler if c == 3 else None)
    else:
        raise ValueError(layout)


    for pair in range(4):
        proj_group(2 * pair, 3)
        proj_group(2 * pair + 1, 3)

    _ob3_cm.__exit__(None, None, None)
    _nrm_cm.__exit__(None, None, None)
    _prp_cm.__exit__(None, None, None)
    _cxp_cm.__exit__(None, None, None)
    _scp_cm.__exit__(None, None, None)
    _const_cm.__exit__(None, None, None)


def _body_v2(nc, xT, wq, wk, wv, wp, tri, ident, outT, tc, state=None):
    """Pair-concurrent attention: heads (2c, 2c+1) processed together.

    Every matmul in the kernel is a K=64 row-tile instruction on PE tiles
    T0 (SBUF partitions 0:63) / T8 (64:127), so the PE never changes
    tiling mode, and back-to-back T0/T8 instructions targeting different
    PSUM banks execute concurrently (~2x stream rate):
      - qkv/proj: contraction split into even/odd 64-row chunks, T0/T8
        halves accumulate into the two halves of a [128,1024] PSUM slot,
        merged by one DVE add on evacuation.
      - scores: head A on T0 -> slot cols [0,512), head B on T8 -> cols
        [512,1024) (different banks); causal mask added to the diagonal
        128-block by split-identity x tri matmuls; ONE exp per (pair,qb,j)
        over a strided [128,2,512-lo] view.
      - ctx: 4 accumulators (A/B x lo/hi k-halves), each PSUM bank touched
        by exactly one PE row-tile; merged in the normalize step.
    Normalize: the softmax denominators are DMA-reshaped [1,1024]->[128,8]
    so the (8-cycle-per-element, single-lane) DVE reciprocal runs on 128
    lanes, then DMA'd back and partition-broadcast.
    Scheduling: qb-outer over q-blocks, pair-inner; qkv blocks for token
    range tb are emitted as PE fillers during qb=tb-1 (deadline-forced),
    proj blocks for tb during qb>=tb+1; ~1 filler per 2 attention steps
    hides the exp latency that the 2-slot scores ring exposes.
    """
    if state is None:
        _cms = [tc.tile_pool(name="const", bufs=1)]
        const = _cms[-1].__enter__()
        state = {
            "_cms": _cms,
            "xT_sb": const.tile([128, 8, S], BF16),
            "wq_sb": const.tile([128, 8, GC], BF16),
            "wk_sb": const.tile([128, 8, GC], BF16),
            "wv_sb": const.tile([128, 8, GC], BF16),
            "wp_sb": const.tile([128, 4, D], BF16),
            "tri_sb": const.tile([128, 128], BF16),
            "ident_sb": const.tile([128, 128], BF16),
            "qT_sb": const.tile([128, 4, S], BF16),
            "kT_sb": const.tile([128, 4, S], BF16),
            "vaug_sb": const.tile([128, 16, HG, 65], BF16),
            "ctxT_sb": const.tile([128, 4, S], BF16),
            "m01_sb": const.tile([128, 128], BF16),
        }
        for nm, mk in (
            ("scp", lambda: tc.tile_pool(name="scp", bufs=2, space="PSUM")),
            ("cxp", lambda: tc.tile_pool(name="cxp", bufs=1, space="PSUM")),
            ("pTp", lambda: tc.tile_pool(name="pTp", bufs=4)),
            ("stp", lambda: tc.tile_pool(name="stp", bufs=2)),
            ("nrm", lambda: tc.tile_pool(name="nrm", bufs=2)),
            ("obp", lambda: tc.tile_pool(name="obp", bufs=3)),
        ):
            cm = mk()
            _cms.append(cm)
            state[nm] = cm.__enter__()
    (xT_sb, wq_sb, wk_sb, wv_sb, wp_sb, tri_sb, ident_sb, qT_sb, kT_sb,
     vaug_sb, ctxT_sb, m01_sb) = (
        state[k] for k in ("xT_sb", "wq_sb", "wk_sb", "wv_sb", "wp_sb",
                           "tri_sb", "ident_sb", "qT_sb", "kT_sb",
                           "vaug_sb", "ctxT_sb", "m01_sb"))
    scp, cxp, pTp, stp, nrm, obp = (
        state[k] for k in ("scp", "cxp", "pTp", "stp", "nrm", "obp"))

    nc.vector.memset(vaug_sb[:, :, :, 64:65], 1.0)
    nc.sync.dma_start(out=tri_sb[:], in_=tri.ap())
    nc.sync.dma_start(out=ident_sb[:], in_=ident.ap())
    # 0/1 causal mask from tri (0 / -1e9): m01 = sign(tri) + 1
    nc.scalar.activation(m01_sb[:], tri_sb[:],
                         mybir.ActivationFunctionType.Sign)
    nc.scalar.add(m01_sb[:], m01_sb[:], 1.0)
    # k/q weights and the tb=0 slice of x first so attention can start ASAP
    for t in range(8):
        nc.sync.dma_start(out=wk_sb[:, t, :], in_=wk.ap()[128 * t:128 * (t + 1), :])
        nc.sync.dma_start(out=wq_sb[:, t, :], in_=wq.ap()[128 * t:128 * (t + 1), :])
    for tb in range(4):
        for t in range(8):
            nc.sync.dma_start(
                out=xT_sb[:, t, 512 * tb:512 * (tb + 1)],
                in_=xT.ap()[128 * t:128 * (t + 1), 512 * tb:512 * (tb + 1)])
        if tb == 0:
            for t in range(8):
                nc.sync.dma_start(
                    out=wv_sb[:, t, :], in_=wv.ap()[128 * t:128 * (t + 1), :])
    for ct in range(4):
        nc.sync.dma_start(out=wp_sb[:, ct, :], in_=wp.ap()[128 * ct:128 * (ct + 1), :])

    # qkv/proj fillers run as plain 128-contraction matmuls (a split-K
    # T0/T8 pair has the same wall time — stream-bound either way — but
    # would need a 2-PSUM-input merge, which DVE can't do in one op)
    def qk_block(which, c, tb):
        w_sb, dst = ((wq_sb, qT_sb), (wk_sb, kT_sb))[which]
        slot = scp.tile([128, 1024], F32, tag="sc", name=f"qk{which}_{c}_{tb}")
        cs = slice(128 * c, 128 * (c + 1))
        ts = slice(512 * tb, 512 * (tb + 1))
        for t in range(8):
            nc.tensor.matmul(
                slot[:, 0:512], lhsT=w_sb[:, t, cs], rhs=xT_sb[:, t, ts],
                start=(t == 0), stop=(t == 7), skip_group_check=True)
        nc.vector.tensor_copy(dst[:, c, ts], slot[:, 0:512])

    def v_block(j):
        slot = scp.tile([128, 1024], F32, tag="sc", name=f"v_{j}")
        js = slice(128 * j, 128 * (j + 1))
        for t in range(8):
            nc.tensor.matmul(
                slot[:, 0:512], lhsT=xT_sb[:, t, js], rhs=wv_sb[:, t, :],
                start=(t == 0), stop=(t == 7), skip_group_check=True)
        nc.vector.tensor_copy(
            vaug_sb[:, j, :, 0:64],
            slot[:, 0:512].rearrange("p (h c) -> p h c", h=HG))

    def proj_block(m, tb):
        slot = scp.tile([128, 1024], F32, tag="sc", name=f"p_{m}_{tb}")
        ms = slice(128 * m, 128 * (m + 1))
        ts = slice(512 * tb, 512 * (tb + 1))
        for ct in range(4):
            nc.tensor.matmul(
                slot[:, 0:512], lhsT=wp_sb[:, ct, ms], rhs=ctxT_sb[:, ct, ts],
                start=(ct == 0), stop=(ct == 3), skip_group_check=True)
        ob = obp.tile([128, 512], F32, tag="ob", name=f"ob_{m}_{tb}")
        nc.vector.tensor_copy(ob[:], slot[:, 0:512])
        nc.sync.dma_start(out=outT.ap()[ms, ts], in_=ob[:])

    # ---- filler queue: (ready_pos, deadline_pos, thunk); pos = 4*qb + c ----
    fillers = []
    for c in range(1, 4):
        fillers.append((0, c, lambda c=c: qk_block(1, c, 0)))
        fillers.append((0, c, lambda c=c: qk_block(0, c, 0)))
    for tb in range(1, 4):
        for c in range(4):
            fillers.append((4 * (tb - 1), 4 * tb + c,
                            lambda c=c, tb=tb: qk_block(1, c, tb)))
            fillers.append((4 * (tb - 1), 4 * tb + c,
                            lambda c=c, tb=tb: qk_block(0, c, tb)))
        for r in range(4):
            fillers.append((4 * (tb - 1), 4 * tb,
                            lambda j=4 * tb + r: v_block(j)))
    for tb in range(4):
        for m in range(8):
            fillers.append((4 * (tb + 1), 99,
                            lambda m=m, tb=tb: proj_block(m, tb)))

    def pop_filler(pos, force=False):
        for i, (rdy, dl, th) in enumerate(fillers):
            if rdy <= pos and (not force or dl <= pos):
                fillers.pop(i)
                th()
                return True
        return False

    def attn_block(c, qb, pos):
        n = 4 * qb + 4
        A, B = 2 * c, 2 * c + 1
        qs0 = 512 * qb
        accs = [cxp.tile([65, 512], F32, tag=t, name=f"{t}_{c}_{qb}")
                for t in ("calo", "cahi", "cblo", "cbhi")]

        def scores_exp(j):
            lo = 128 * (j - 4 * qb) if j >= 4 * qb else 0
            diag = (j >= 4 * qb) and V2_MASK != "dve"
            slot = scp.tile([128, 1024], F32, tag="sc", name=f"s_{c}_{qb}_{j}")
            js = slice(128 * j, 128 * (j + 1))
            nc.tensor.matmul(
                slot[:, lo:512], lhsT=kT_sb[0:64, c, js],
                rhs=qT_sb[0:64, c, qs0 + lo:qs0 + 512],
                start=True, stop=not diag, skip_group_check=True)
            nc.tensor.matmul(
                slot[:, 512 + lo:1024], lhsT=kT_sb[64:128, c, js],
                rhs=qT_sb[64:128, c, qs0 + lo:qs0 + 512],
                start=True, stop=not diag, skip_group_check=True)
            if diag and V2_MASK == "split":
                # add tri (0/-1e9) to the diagonal 128-block of both heads;
                # T0 pair first, then T8 pair (bank-overlap safe: a T8 mask
                # starts only after the same-bank T0 mask has finished)
                for h0, last in ((0, False), (64, True)):
                    nc.tensor.matmul(
                        slot[:, lo:lo + 128], lhsT=ident_sb[h0:h0 + 64, :],
                        rhs=tri_sb[h0:h0 + 64, :],
                        start=False, stop=last, skip_group_check=True)
                    nc.tensor.matmul(
                        slot[:, 512 + lo:512 + lo + 128],
                        lhsT=ident_sb[h0:h0 + 64, :], rhs=tri_sb[h0:h0 + 64, :],
                        start=False, stop=last, skip_group_check=True)
            elif diag:
                for off, last in ((0, False), (512, True)):
                    nc.tensor.matmul(
                        slot[:, off + lo:off + lo + 128], lhsT=ident_sb[:],
                        rhs=tri_sb[:],
                        start=False, stop=last, skip_group_check=True)
            pT = pTp.tile([128, 1024], BF16, tag="pt", name=f"pT_{c}_{qb}_{j}")
            if V2_EXP == "fused" or lo == 0:
                vi = slot[:].rearrange("p (b w) -> p b w", b=2)[:, :, lo:512]
                vo = pT[:].rearrange("p (b w) -> p b w", b=2)[:, :, lo:512]
                nc.scalar.activation(vo, vi, EXP, scale=SCALE)
            else:
                nc.scalar.activation(pT[:, lo:512], slot[:, lo:512],
                                     EXP, scale=SCALE)
                nc.scalar.activation(pT[:, 512 + lo:1024],
                                     slot[:, 512 + lo:1024], EXP, scale=SCALE)
            if j >= 4 * qb and V2_MASK == "dve":
                # zero the strict upper triangle of the diagonal 128-block
                # (probs layout is [k, q]: valid iff q >= k)
                nc.vector.tensor_mul(
                    pT[:, lo:lo + 128], pT[:, lo:lo + 128], m01_sb[:])
                nc.vector.tensor_mul(
                    pT[:, 512 + lo:512 + lo + 128],
                    pT[:, 512 + lo:512 + lo + 128], m01_sb[:])
            return pT, lo

        def ctx(j, pT, lo):
            first, last = (j == 0), (j == n - 1)
            if V2_CTX == "split":
                for i, (h, p0) in enumerate(((A, 0), (A, 64), (B, 0), (B, 64))):
                    off = 0 if h == A else 512
                    nc.tensor.matmul(
                        accs[i][:, lo:512],
                        lhsT=vaug_sb[p0:p0 + 64, j, h, :],
                        rhs=pT[p0:p0 + 64, off + lo:off + 512],
                        start=first, stop=last, skip_group_check=True)
            else:
                for i, h, off in ((0, A, 0), (2, B, 512)):
                    nc.tensor.matmul(
                        accs[i][:, lo:512],
                        lhsT=vaug_sb[:, j, h, :],
                        rhs=pT[:, off + lo:off + 512],
                        start=first, stop=last, skip_group_check=True)

        prev = None
        for j in range(n):
            if j % 2 == 1:
                pop_filler(pos)
            pT, lo = scores_exp(j)
            if prev is not None:
                ctx(*prev)
            prev = (j, pT, lo)
        # drain: one filler covers the last exp's latency, then final ctx
        pop_filler(pos)
        ctx(*prev)

        # ---- normalize both heads of the pair for this q-block ----
        st = stp.tile([65, 1024], F32, tag="st", name=f"st_{c}_{qb}")
        # DVE can read only ONE PSUM operand per op: copy then add
        nc.vector.tensor_copy(st[:, 0:512], accs[0][:])
        nc.vector.tensor_copy(st[:, 512:1024], accs[2][:])
        if V2_CTX == "split":
            nc.vector.tensor_add(st[:, 0:512], st[:, 0:512], accs[1][:])
            nc.vector.tensor_add(st[:, 512:1024], st[:, 512:1024], accs[3][:])
        rec = nrm.tile([1, 1024], F32, tag="rec", name=f"rec_{c}_{qb}")
        if V2_RECIP == "dma":
            # reciprocal of the [1,1024] sums row is 8 cyc/elem on ONE
            # lane; DMA-reshape to [128,8] puts it on 128 lanes
            zT = nrm.tile([128, 8], F32, tag="zt", name=f"zt_{c}_{qb}")
            nc.sync.dma_start(out=zT[:], in_=st[64:65, :])
            zR = nrm.tile([128, 8], F32, tag="zr", name=f"zr_{c}_{qb}")
            nc.vector.reciprocal(zR[:], zT[:])
            nc.sync.dma_start(out=rec[:], in_=zR[:])
        else:
            nc.vector.reciprocal(rec[:], st[64:65, :])
        bc = nrm.tile([64, 1024], F32, tag="bc", name=f"bc_{c}_{qb}")
        nc.gpsimd.partition_broadcast(bc[:], rec[:])
        cols = slice(qs0, qs0 + 512)
        nc.vector.tensor_mul(ctxT_sb[0:64, c, cols], st[0:64, 0:512], bc[:, 0:512])
        stgb = nrm.tile([64, 512], BF16, tag="stgb", name=f"stgb_{c}_{qb}")
        nc.vector.tensor_mul(stgb[:], st[0:64, 512:1024], bc[:, 512:1024])
        nc.sync.dma_start(out=ctxT_sb[64:128, c, cols], in_=stgb[:])

    # upfront work: k/q for pair 0 and all tb=0 v tiles
    qk_block(1, 0, 0)
    qk_block(0, 0, 0)
    for j in range(4):
        v_block(j)

    for qb in range(4):
        for c in range(4):
            pos = 4 * qb + c
            if (qb, c) != (0, 0):
                # k/q for this pair's q-block + anything past deadline
                while pop_filler(pos, force=True):
                    pass
                pop_filler(pos)
            attn_block(c, qb, pos)
    while pop_filler(99):
        pass

    return state


def _v2_close(state):
    for cm in reversed(state["_cms"]):
        cm.__exit__(None, None, None)


_CACHED = {}


def _build(reps=1, layout="fill", use_bias=True):
    key = (reps, layout, use_bias, V2_RECIP, V2_EXP, V2_CTX, V2_MASK,
           P_MASK, P_RECIP)
    if key in _CACHED:
        return _CACHED[key]
    nc = bacc.Bacc()
    xT = nc.dram_tensor("xT", [D, S], BF16, kind="ExternalInput")
    wq = nc.dram_tensor("wq", [D, GC], BF16, kind="ExternalInput")
    wk = nc.dram_tensor("wk", [D, GC], BF16, kind="ExternalInput")
    wv = nc.dram_tensor("wv", [D, GC], BF16, kind="ExternalInput")
    wp = nc.dram_tensor("wp", [GC, D], BF16, kind="ExternalInput")
    bqkv = None
    if layout not in ("v2", "p3"):
        bqkv = nc.dram_tensor("bqkv", [1, 3 * GC], BF16, kind="ExternalInput")
    tri = nc.dram_tensor("tri", [128, 128], BF16, kind="ExternalInput")
    ident = nc.dram_tensor("ident", [128, 128], BF16, kind="ExternalInput")
    outT = nc.dram_tensor("outT", [D, S], F32, kind="ExternalOutput")
    with tile.TileContext(nc) as tc:
        st = None
        for _ in range(reps):
            if layout == "v2":
                st = _body_v2(nc, xT, wq, wk, wv, wp, tri, ident, outT, tc,
                              state=st)
            else:
                _body(nc, xT, wq, wk, wv, wp, bqkv, tri, ident, outT, tc,
                      layout=layout, use_bias=use_bias)
        if st is not None:
            _v2_close(st)
    nc.compile()
    _CACHED[key] = nc
    return nc


def make_in_maps(x, W_attn, b_attn, W_proj):
    bf = ml_dtypes.bfloat16
    tri_np = np.where(np.arange(128)[None, :] >= np.arange(128)[:, None],
                      np.float32(0.0), np.float32(-1e9)).astype(bf)
    ident_np = np.eye(128, dtype=np.float32).astype(bf)
    in_maps = []
    for core in range(N_CORES):
        b, g = divmod(core, 2)
        cols = slice(GC * g, GC * (g + 1))
        in_maps.append({
            "xT": np.ascontiguousarray(x[b].T).astype(bf),
            "wq": np.ascontiguousarray(W_attn[:, cols]).astype(bf),
            "wk": np.ascontiguousarray(W_attn[:, D:][:, cols]).astype(bf),
            "wv": np.ascontiguousarray(W_attn[:, 2 * D:][:, cols]).astype(bf),
            "wp": np.ascontiguousarray(W_proj[cols, :]).astype(bf),
            "bqkv": np.concatenate(
                [b_attn[cols], b_attn[D:][cols], b_attn[2 * D:][cols]]
            ).reshape(1, 3 * GC).astype(bf),
            "tri": tri_np,
            "ident": ident_np,
        })
    return in_maps


LAYOUT = "pipe"  # current best layout; kernel() and test.py both use it
P_MASK = "pe"    # "pe" (ident x tri matmul) | "dve" (0/1 multiply on pT)
P_RECIP = "dve"  # "dve" (1-lane reciprocal) | "dma" (128-lane via reshape)
P_PROJ_EVAC = "act"  # "act" (scalar.copy) | "dve" (vector.tensor_copy)
V2_RECIP = "dma"   # "dma" (128-lane via DMA reshape) | "dve" (1-lane direct)
V2_EXP = "fused"   # "fused" (one strided exp per pair step) | "split"
V2_CTX = "split"   # "split" (4x 64-contract T0/T8) | "full" (2x 128-contract)
# "dve": multiply pT's diagonal block by a 0/1 mask on DVE (no PE mask work)
# "full": baseline-style 128-contract ident x tri matmul into PSUM
# "split": ident halves T0/T8 — BROKEN on HW (mixing PE row-tiles inside
#          one PSUM accumulation group fails device execution)
V2_MASK = "dve"


def kernel(x, W_attn, b_attn, W_proj, b_proj, _run_kwargs=None):
    x = np.asarray(x)
    W_attn = np.asarray(W_attn)
    b_attn = np.asarray(b_attn)
    W_proj = np.asarray(W_proj)
    b_proj = np.asarray(b_proj)

    use_bias = bool(np.any(b_attn))
    layout = "fill" if (use_bias and LAYOUT in ("v2", "p3")) else LAYOUT
    nc = _build(layout=layout, use_bias=use_bias)
    in_maps = make_in_maps(x, W_attn, b_attn, W_proj)
    declared = {
        alloc.memorylocations[0].name
        for alloc in nc.m.functions[0].allocations
        if isinstance(alloc, mybir.MemoryLocationSet)
        and alloc.kind == "ExternalInput"
    }
    in_maps = [{k: v for k, v in m.items() if k in declared} for m in in_maps]

    res = run_bass_kernel_spmd(
        nc, in_maps, core_ids=list(range(N_CORES)), **(_run_kwargs or {}))

    out = np.empty((B, S, D), np.float32)
    for b in range(B):
        acc = res.results[2 * b]["outT"] + res.results[2 * b + 1]["outT"]
        out[b] = acc.T + b_proj[None, :].astype(np.float32)
    if _run_kwargs:
        kernel.last_results = res
    return out

